# revision 2
# baseline (speedup 1.0000x reference)
"""Mixtral MoE (top-2 of 8 experts, GLU) on 8 Trainium2 cores.

Strategy (expert-parallel, MegaBlocks-style host dispatch):
  - Host computes the router exactly (fp32, same ops as the reference) and
    flattens the T*K = 16384 (token, expert, weight) assignments.
  - Assignments are grouped by expert and chopped into chunks of <= 512
    tokens; chunks are bin-packed onto 8 cores x 5 segment slots (always
    feasible: sum_e ceil(c_e/512) <= 39 <= 40), so every core does the
    same fixed amount of work regardless of router skew.
  - Each core's device inputs are host-assembled: gathered token blocks,
    per-segment pre-transposed bf16 weights (w1^T, v1^T in [H,F] layout,
    w2 in [F,H] layout), and per-token combine coefficients.
  - The device kernel is fully static: per segment, stream F-tiles of the
    three weight matrices, compute hmid^T = silu(w1 x^T) * (v1 x^T),
    then accumulate y = hmid @ w2 over F-tiles, scale by coef, write out.
    All matmuls are bf16 with fp32 accumulation.
  - Host scatter-adds the per-segment outputs into the full [T, H] output.
"""

import numpy as np
import ml_dtypes

B, S, H, F, E, TOPK = 4, 2048, 1024, 3584, 8, 2
T = B * S
NCORES = 8
SEG_TOK = 512          # tokens per segment (capacity)
NSEG = 5               # segments per core
NBPS = SEG_TOK // 128  # 128-token sub-blocks per segment
NFT = 7                # F tiles
FT = F // NFT          # 512
BF16 = ml_dtypes.bfloat16

_compiled = {}


def _build_nc():
    import concourse.tile as tile
    import concourse.mybir as mybir
    from concourse import bacc

    nc = bacc.Bacc("TRN2", target_bir_lowering=False, debug=False,
                   num_devices=NCORES)
    xt = nc.dram_tensor("xt", [NSEG, 128, 8, SEG_TOK], mybir.dt.bfloat16,
                        kind="ExternalInput")
    w1t = nc.dram_tensor("w1t", [NSEG, NFT, 128, 8, FT], mybir.dt.bfloat16,
                         kind="ExternalInput")
    v1t = nc.dram_tensor("v1t", [NSEG, NFT, 128, 8, FT], mybir.dt.bfloat16,
                         kind="ExternalInput")
    w2 = nc.dram_tensor("w2", [NSEG, NFT, 128, 4, H], mybir.dt.bfloat16,
                        kind="ExternalInput")
    coef = nc.dram_tensor("coef", [NSEG, 128, NBPS], mybir.dt.float32,
                          kind="ExternalInput")
    yout = nc.dram_tensor("yout", [NSEG, 128, NBPS, H], mybir.dt.float32,
                          kind="ExternalOutput")

    FC = FT // 128  # 128-row f chunks per f-tile
    NH = H // 512   # 512-wide h chunks

    with tile.TileContext(nc) as tc:
        with (
            tc.tile_pool(name="xpool", bufs=2) as xpool,
            tc.tile_pool(name="wpool", bufs=2) as wpool,
            tc.tile_pool(name="hpool", bufs=2) as hpool,
            tc.tile_pool(name="spool", bufs=2) as spool,
            tc.tile_pool(name="opool", bufs=2) as opool,
            tc.tile_pool(name="cpool", bufs=2) as cpool,
            tc.tile_pool(name="ps1", bufs=2, space="PSUM") as ps1,
            tc.tile_pool(name="ps2", bufs=2, space="PSUM") as ps2,
            tc.tile_pool(name="pso", bufs=2, space="PSUM") as psop,
        ):
            for s in range(NSEG):
                xts = xpool.tile([128, 8, SEG_TOK], mybir.dt.bfloat16)
                nc.sync.dma_start(xts[:], xt[s])
                coefs = cpool.tile([128, NBPS], mybir.dt.float32)
                nc.sync.dma_start(coefs[:], coef[s])
                oacc = opool.tile([128, NBPS, H], mybir.dt.float32)

                for ft in range(NFT):
                    w1s = wpool.tile([128, 8, FT], mybir.dt.bfloat16, tag="w1s")
                    nc.sync.dma_start(w1s[:], w1t[s, ft])
                    v1s = wpool.tile([128, 8, FT], mybir.dt.bfloat16, tag="v1s")
                    nc.sync.dma_start(v1s[:], v1t[s, ft])
                    w2s = wpool.tile([128, 4, H], mybir.dt.bfloat16, tag="w2s")
                    nc.sync.dma_start(w2s[:], w2[s, ft])

                    hmid = hpool.tile([128, FC, SEG_TOK], mybir.dt.bfloat16)
                    for fc in range(FC):
                        p1 = ps1.tile([128, SEG_TOK], mybir.dt.float32)
                        p2 = ps2.tile([128, SEG_TOK], mybir.dt.float32)
                        for hs in range(8):
                            nc.tensor.matmul(
                                p1[:], w1s[:, hs, fc * 128:(fc + 1) * 128],
                                xts[:, hs, :], start=(hs == 0), stop=(hs == 7))
                        for hs in range(8):
                            nc.tensor.matmul(
                                p2[:], v1s[:, hs, fc * 128:(fc + 1) * 128],
                                xts[:, hs, :], start=(hs == 0), stop=(hs == 7))
                        sil = spool.tile([128, SEG_TOK], mybir.dt.float32)
                        nc.scalar.activation(sil[:], p1[:],
                                             mybir.ActivationFunctionType.Silu)
                        nc.vector.tensor_mul(hmid[:, fc, :], sil[:], p2[:])

                    for m in range(NBPS):
                        for n in range(NH):
                            po = psop.tile([128, 512], mybir.dt.float32)
                            for fc in range(FC):
                                nc.tensor.matmul(
                                    po[:], hmid[:, fc, m * 128:(m + 1) * 128],
                                    w2s[:, fc, n * 512:(n + 1) * 512],
                                    start=(fc == 0), stop=(fc == FC - 1))
                            osl = oacc[:, m, n * 512:(n + 1) * 512]
                            if ft == 0:
                                nc.scalar.copy(osl, po[:])
                            else:
                                nc.vector.tensor_add(osl, osl, po[:])

                for m in range(NBPS):
                    nc.vector.tensor_scalar_mul(
                        oacc[:, m, :], oacc[:, m, :], coefs[:, m:m + 1])
                nc.sync.dma_start(yout[s], oacc[:])

    nc.compile()
    return nc


def _get_nc():
    if "nc" not in _compiled:
        _compiled["nc"] = _build_nc()
    return _compiled["nc"]


def _route(x, router_w):
    """Top-2 router, matching the reference (jax on CPU if available)."""
    try:
        import jax
        import jax.numpy as jnp
        cpu = jax.devices("cpu")[0]
        with jax.default_device(cpu):
            xl = jax.device_put(jnp.asarray(x), cpu)
            rw = jax.device_put(jnp.asarray(router_w), cpu)
            logits = xl @ rw.T
            scores = jax.nn.softmax(logits.astype(jnp.float32), axis=-1)
            ew, ei = jax.lax.top_k(scores, TOPK)
            ew = ew / ew.sum(axis=-1, keepdims=True)
            return np.asarray(ew, np.float32), np.asarray(ei, np.int64)
    except Exception:
        logits = x.astype(np.float32) @ router_w.astype(np.float32).T
        m = logits.max(axis=-1, keepdims=True)
        p = np.exp(logits - m)
        scores = (p / p.sum(axis=-1, keepdims=True)).astype(np.float32)
        i1 = scores.argmax(axis=-1)
        s2 = scores.copy()
        s2[np.arange(T), i1] = -np.inf
        i2 = s2.argmax(axis=-1)
        wa = scores[np.arange(T), i1]
        wb = scores[np.arange(T), i2]
        tot = wa + wb
        ew = np.stack([wa / tot, wb / tot], axis=-1).astype(np.float32)
        ei = np.stack([i1, i2], axis=-1).astype(np.int64)
        return ew, ei


def _pack(ei, ew):
    """Group assignments by expert, chop into <=SEG_TOK chunks, bin-pack
    onto NCORES x NSEG slots balancing token counts.

    Returns per-core list of (expert, token_ids, weights)."""
    flat_e = ei.ravel()
    flat_w = ew.ravel().astype(np.float32)
    order = np.argsort(flat_e, kind="stable")
    toks = (order // TOPK).astype(np.int64)
    ws = flat_w[order]
    counts = np.bincount(flat_e, minlength=E)
    starts = np.concatenate([[0], np.cumsum(counts)])

    chunks = []
    for e in range(E):
        lo, hi = int(starts[e]), int(starts[e + 1])
        for c0 in range(lo, hi, SEG_TOK):
            c1 = min(c0 + SEG_TOK, hi)
            chunks.append((e, toks[c0:c1], ws[c0:c1]))
    assert len(chunks) <= NCORES * NSEG, f"{len(chunks)} chunks > capacity"

    chunks.sort(key=lambda c: -len(c[1]))
    core_loads = [0] * NCORES
    core_segs = [[] for _ in range(NCORES)]
    for ch in chunks:
        cands = [c for c in range(NCORES) if len(core_segs[c]) < NSEG]
        c = min(cands, key=lambda i: core_loads[i])
        core_segs[c].append(ch)
        core_loads[c] += len(ch[1])
    return core_segs


def _to_bf16(a):
    """Fast float32 -> bfloat16 with round-to-nearest-even."""
    u = np.ascontiguousarray(a, np.float32).view(np.uint32)
    r = ((u + np.uint32(0x7FFF) + ((u >> np.uint32(16)) & np.uint32(1)))
         >> np.uint32(16)).astype(np.uint16)
    return r.view(BF16)


def _prep_weights(w1, v1, w2):
    """Per-expert device layouts (bf16).

    w1t/v1t: [E][NFT,128,8,FT]  elem [ft,p,hs,f] = W[ft*FT+f, hs*128+p]
    w2     : [E][NFT,128,4,H]   elem [ft,p,fc,h] = w2[ft*FT+fc*128+p, h]
    """
    w1t, v1t, w2d = [], [], []
    for e in range(E):
        for src, dst in ((w1, w1t), (v1, v1t)):
            a = _to_bf16(src[e])                      # [F, H]
            a = np.ascontiguousarray(a.T)             # [H, F]
            a = a.reshape(8, 128, NFT, FT).transpose(2, 1, 0, 3)
            dst.append(np.ascontiguousarray(a))
        b = _to_bf16(w2[e])                           # [F, H]
        b = b.reshape(NFT, 4, 128, H).transpose(0, 2, 1, 3)
        w2d.append(np.ascontiguousarray(b))
    return w1t, v1t, w2d


def _forward(hidden_states, router_w, w1, v1, w2, trace=False):
    from concourse.bass_utils import run_bass_kernel_spmd

    x = np.ascontiguousarray(np.asarray(hidden_states, np.float32)).reshape(T, H)
    router_w = np.asarray(router_w, np.float32)
    w1 = np.asarray(w1, np.float32)
    v1 = np.asarray(v1, np.float32)
    w2 = np.asarray(w2, np.float32)

    ew, ei = _route(x, router_w)
    core_segs = _pack(ei, ew)
    w1t_pre, v1t_pre, w2_pre = _prep_weights(w1, v1, w2)
    xbf = _to_bf16(x)  # [T, H] bf16

    in_maps = []
    for c in range(NCORES):
        xt_np = np.zeros((NSEG, 128, 8, SEG_TOK), BF16)
        w1t_np = np.zeros((NSEG, NFT, 128, 8, FT), BF16)
        v1t_np = np.zeros((NSEG, NFT, 128, 8, FT), BF16)
        w2_np = np.zeros((NSEG, NFT, 128, 4, H), BF16)
        coef_np = np.zeros((NSEG, 128, NBPS), np.float32)
        for s, (e, ids, ws) in enumerate(core_segs[c]):
            L = len(ids)
            xg = np.ascontiguousarray(xbf[ids].T)     # [H, L]
            xt_np[s, :, :, :L] = xg.reshape(8, 128, L).transpose(1, 0, 2)
            wpad = np.zeros(SEG_TOK, np.float32)
            wpad[:L] = ws
            coef_np[s] = wpad.reshape(NBPS, 128).T
            w1t_np[s] = w1t_pre[e]
            v1t_np[s] = v1t_pre[e]
            w2_np[s] = w2_pre[e]
        in_maps.append({"xt": xt_np, "w1t": w1t_np, "v1t": v1t_np,
                        "w2": w2_np, "coef": coef_np})

    nc = _get_nc()
    if trace:
        _install_profile_shim()
    res = run_bass_kernel_spmd(nc, in_maps, list(range(NCORES)), trace=trace)

    out = np.zeros((T, H), np.float32)
    for c in range(NCORES):
        y = res.results[c]["yout"]  # [NSEG, 128, NBPS, H]
        for s, (e, ids, ws) in enumerate(core_segs[c]):
            L = len(ids)
            yseg = y[s].transpose(1, 0, 2).reshape(SEG_TOK, H)[:L]
            out[ids] += yseg
    return out.reshape(B, S, H), res


def kernel(hidden_states, router_w, w1, v1, w2):
    out, _ = _forward(hidden_states, router_w, w1, v1, w2, trace=False)
    return out


def _install_profile_shim():
    """The agent image's antenv lacks axon_hooks; register the NTFF
    profile hook from trn_agent_boot so trace=True works."""
    import sys
    import types
    if "antenv.axon_hooks" in sys.modules:
        return
    holder = {}
    mod = types.ModuleType("antenv.axon_hooks")
    mod.set_axon_ntff_profile_hook = lambda h: holder.__setitem__("h", h)
    mod.get_axon_ntff_profile_hook = lambda: holder.get("h")
    sys.modules["antenv.axon_hooks"] = mod
    try:
        from trn_agent_boot.trn_boot import _ntff_profile_via_ctypes
        hook = _ntff_profile_via_ctypes("/opt/axon/libaxon_pjrt.so")
        mod.set_axon_ntff_profile_hook(hook)
    except Exception as exc:  # pragma: no cover
        print(f"profile shim failed: {exc}")


# revision 3
# speedup vs baseline: 1.1337x; 1.1337x over previous
"""Mixtral MoE (top-2 of 8 experts, GLU) on 8 Trainium2 cores.

Strategy (expert-parallel, MegaBlocks-style host dispatch):
  - Host computes the router exactly (fp32, same ops as the reference) and
    flattens the T*K = 16384 (token, expert, weight) assignments.
  - Assignments are grouped by expert and packed into per-core single-expert
    "segments" chosen from a small list of static slot-size templates; the
    best (smallest-capacity) feasible template for the actual routing is
    selected at runtime and its kernel compiled once (cached). The fallback
    template [512]*5 is always feasible (sum_e ceil(c_e/512) <= 39 <= 40).
  - Each core's device inputs are host-assembled: gathered token blocks,
    per-segment pre-transposed bf16 weights, and per-token combine
    coefficients.
  - The device kernel is fully static: per segment, stream F-tiles of the
    three weight matrices, compute hmid^T = silu(w1 x^T) * (v1 x^T),
    then accumulate y = hmid @ w2 over F-tiles, scale by coef, write out.
    All matmuls are bf16 with fp32 accumulation.
  - Host scatter-adds the per-segment outputs into the full [T, H] output.
"""

import numpy as np
import ml_dtypes

B, S, H, F, E, TOPK = 4, 2048, 1024, 3584, 8, 2
T = B * S
NCORES = 8
NFT = 7                # F tiles
FT = F // NFT          # 512
BF16 = ml_dtypes.bfloat16

# Candidate per-core segment templates, preferred order (lower capacity
# first; ties prefer fewer segments = less weight streaming).
TEMPLATES = [
    (512, 512, 512, 512),            # 2048
    (1024, 1024),                    # 2048
    (512, 512, 512, 512, 128),       # 2176
    (512, 512, 512, 384, 256),       # 2176
    (768, 768, 768),                 # 2304
    (512, 512, 512, 512, 256),       # 2304
    (512, 512, 512, 384, 384),       # 2304
    (384, 384, 384, 384, 384, 384),  # 2304
    (512, 512, 512, 512, 384),       # 2432
    (512, 512, 512, 512, 512),       # 2560 — always feasible
]

_compiled = {}


# --------------------------------------------------------------------------
# device kernel
# --------------------------------------------------------------------------

def _build_nc(template):
    import concourse.tile as tile
    import concourse.mybir as mybir
    from concourse import bacc

    cap = sum(template)           # tokens per core
    nseg = len(template)
    offs = np.concatenate([[0], np.cumsum(template)]).astype(int)
    NH = H // 512                 # 512-wide h chunks
    FC = FT // 128                # 128-row f chunks per f-tile

    nc = bacc.Bacc("TRN2", target_bir_lowering=False, debug=False,
                   num_devices=NCORES)
    xt = nc.dram_tensor("xt", [128, 8, cap], mybir.dt.bfloat16,
                        kind="ExternalInput")
    w1t = nc.dram_tensor("w1t", [nseg, NFT, 128, 8, FT], mybir.dt.bfloat16,
                         kind="ExternalInput")
    v1t = nc.dram_tensor("v1t", [nseg, NFT, 128, 8, FT], mybir.dt.bfloat16,
                         kind="ExternalInput")
    w2 = nc.dram_tensor("w2", [nseg, NFT, 128, 4, H], mybir.dt.bfloat16,
                        kind="ExternalInput")
    coef = nc.dram_tensor("coef", [128, cap // 128], mybir.dt.float32,
                          kind="ExternalInput")
    yout = nc.dram_tensor("yout", [128, cap // 128, H], mybir.dt.float32,
                          kind="ExternalOutput")

    with tile.TileContext(nc) as tc:
        with (
            tc.tile_pool(name="xpool", bufs=2) as xpool,
            tc.tile_pool(name="wpool", bufs=2) as wpool,
            tc.tile_pool(name="hpool", bufs=2) as hpool,
            tc.tile_pool(name="spool", bufs=2) as spool,
            tc.tile_pool(name="opool", bufs=2) as opool,
            tc.tile_pool(name="cpool", bufs=1) as cpool,
            tc.tile_pool(name="ps1", bufs=2, space="PSUM") as ps1,
            tc.tile_pool(name="ps2", bufs=2, space="PSUM") as ps2,
            tc.tile_pool(name="pso", bufs=2, space="PSUM") as psop,
        ):
            coefs = cpool.tile([128, cap // 128], mybir.dt.float32)
            nc.sync.dma_start(coefs[:], coef[:])
            for s in range(nseg):
                st = template[s]
                moff = offs[s] // 128       # token-block offset
                nm = st // 128              # 128-token sub-blocks
                # stage-1 token chunks (<=512 each)
                tchunks = []
                t0 = 0
                while t0 < st:
                    tl = min(512, st - t0)
                    tchunks.append((t0, tl))
                    t0 += tl

                xts = xpool.tile([128, 8, st], mybir.dt.bfloat16, tag="xts")
                nc.sync.dma_start(xts[:], xt[:, :, offs[s]:offs[s + 1]])
                oacc = opool.tile([128, nm, H], mybir.dt.float32, tag="oacc")

                for ft in range(NFT):
                    w1s = wpool.tile([128, 8, FT], mybir.dt.bfloat16, tag="w1s")
                    nc.sync.dma_start(w1s[:], w1t[s, ft])
                    v1s = wpool.tile([128, 8, FT], mybir.dt.bfloat16, tag="v1s")
                    nc.sync.dma_start(v1s[:], v1t[s, ft])
                    w2s = wpool.tile([128, 4, H], mybir.dt.bfloat16, tag="w2s")
                    nc.sync.dma_start(w2s[:], w2[s, ft])

                    hmid = hpool.tile([128, FC, st], mybir.dt.bfloat16,
                                      tag="hmid")
                    for fc in range(FC):
                        fsl = slice(fc * 128, (fc + 1) * 128)
                        for (t0, tl) in tchunks:
                            p1 = ps1.tile([128, 512], mybir.dt.float32)
                            p2 = ps2.tile([128, 512], mybir.dt.float32)
                            for hs in range(8):
                                nc.tensor.matmul(
                                    p1[:, :tl], w1s[:, hs, fsl],
                                    xts[:, hs, t0:t0 + tl],
                                    start=(hs == 0), stop=(hs == 7))
                            for hs in range(8):
                                nc.tensor.matmul(
                                    p2[:, :tl], v1s[:, hs, fsl],
                                    xts[:, hs, t0:t0 + tl],
                                    start=(hs == 0), stop=(hs == 7))
                            sil = spool.tile([128, 512], mybir.dt.float32)
                            nc.scalar.activation(
                                sil[:, :tl], p1[:, :tl],
                                mybir.ActivationFunctionType.Silu)
                            nc.vector.tensor_mul(
                                hmid[:, fc, t0:t0 + tl], sil[:, :tl],
                                p2[:, :tl])

                    for m in range(nm):
                        msl = slice(m * 128, (m + 1) * 128)
                        for n in range(NH):
                            nsl = slice(n * 512, (n + 1) * 512)
                            po = psop.tile([128, 512], mybir.dt.float32)
                            for fc in range(FC):
                                nc.tensor.matmul(
                                    po[:], hmid[:, fc, msl], w2s[:, fc, nsl],
                                    start=(fc == 0), stop=(fc == FC - 1))
                            osl = oacc[:, m, nsl]
                            if ft == 0:
                                nc.scalar.copy(osl, po[:])
                            else:
                                nc.vector.tensor_add(osl, osl, po[:])

                for m in range(nm):
                    nc.vector.tensor_scalar_mul(
                        oacc[:, m, :], oacc[:, m, :],
                        coefs[:, moff + m:moff + m + 1])
                nc.sync.dma_start(yout[:, moff:moff + nm, :], oacc[:])

    nc.compile()
    return nc


def _get_nc(template):
    if template not in _compiled:
        _compiled[template] = _build_nc(template)
    return _compiled[template]


# --------------------------------------------------------------------------
# host side: routing, packing, layout
# --------------------------------------------------------------------------

def _route(x, router_w):
    """Top-2 router, matching the reference (jax on CPU if available)."""
    try:
        import jax
        import jax.numpy as jnp
        cpu = jax.devices("cpu")[0]
        with jax.default_device(cpu):
            xl = jax.device_put(jnp.asarray(x), cpu)
            rw = jax.device_put(jnp.asarray(router_w), cpu)
            logits = xl @ rw.T
            scores = jax.nn.softmax(logits.astype(jnp.float32), axis=-1)
            ew, ei = jax.lax.top_k(scores, TOPK)
            ew = ew / ew.sum(axis=-1, keepdims=True)
            return np.asarray(ew, np.float32), np.asarray(ei, np.int64)
    except Exception:
        logits = x.astype(np.float32) @ router_w.astype(np.float32).T
        m = logits.max(axis=-1, keepdims=True)
        p = np.exp(logits - m)
        scores = (p / p.sum(axis=-1, keepdims=True)).astype(np.float32)
        i1 = scores.argmax(axis=-1)
        s2 = scores.copy()
        s2[np.arange(T), i1] = -np.inf
        i2 = s2.argmax(axis=-1)
        wa = scores[np.arange(T), i1]
        wb = scores[np.arange(T), i2]
        tot = wa + wb
        ew = np.stack([wa / tot, wb / tot], axis=-1).astype(np.float32)
        ei = np.stack([i1, i2], axis=-1).astype(np.int64)
        return ew, ei


def _try_pack(template, counts):
    """Greedy bin-pack: experts (desc count) onto 8 copies of `template`.
    Each slot holds tokens of a single expert. Returns per-core slot
    assignment [{slot_idx: (expert, n_tokens)}] or None if infeasible."""
    slots = []  # (size, core, slot_idx)
    for c in range(NCORES):
        for i, sz in enumerate(template):
            slots.append([sz, c, i, None, 0])  # size, core, idx, expert, used
    free = sorted(range(len(slots)), key=lambda i: -slots[i][0])

    for e in np.argsort(-counts):
        rem = int(counts[e])
        while rem > 0:
            # last piece: smallest free slot that fits it, else largest
            fit = [i for i in free if slots[i][0] >= rem]
            if fit:
                pick = min(fit, key=lambda i: slots[i][0])
            elif free:
                pick = free[0]
            else:
                return None
            free.remove(pick)
            take = min(rem, slots[pick][0])
            slots[pick][3] = int(e)
            slots[pick][4] = take
            rem -= take
    per_core = [dict() for _ in range(NCORES)]
    for sz, c, i, e, used in slots:
        if e is not None:
            per_core[c][i] = (e, used)
    return per_core


def _select_template(counts):
    for tpl in TEMPLATES:
        pack = _try_pack(tpl, counts)
        if pack is not None:
            return tpl, pack
    raise AssertionError("no feasible template (impossible)")


def _to_bf16(a):
    """Fast float32 -> bfloat16 with round-to-nearest-even."""
    u = np.ascontiguousarray(a, np.float32).view(np.uint32)
    r = ((u + np.uint32(0x7FFF) + ((u >> np.uint32(16)) & np.uint32(1)))
         >> np.uint32(16)).astype(np.uint16)
    return r.view(BF16)


def _prep_weights(w1, v1, w2):
    """Per-expert device layouts (bf16).

    w1t/v1t: [E][NFT,128,8,FT]  elem [ft,p,hs,f] = W[ft*FT+f, hs*128+p]
    w2     : [E][NFT,128,4,H]   elem [ft,p,fc,h] = w2[ft*FT+fc*128+p, h]
    """
    w1t, v1t, w2d = [], [], []
    for e in range(E):
        for src, dst in ((w1, w1t), (v1, v1t)):
            a = _to_bf16(src[e])                      # [F, H]
            a = np.ascontiguousarray(a.T)             # [H, F]
            a = a.reshape(8, 128, NFT, FT).transpose(2, 1, 0, 3)
            dst.append(np.ascontiguousarray(a))
        b = _to_bf16(w2[e])                           # [F, H]
        b = b.reshape(NFT, 4, 128, H).transpose(0, 2, 1, 3)
        w2d.append(np.ascontiguousarray(b))
    return w1t, v1t, w2d


def _forward(hidden_states, router_w, w1, v1, w2, trace=False):
    from concourse.bass_utils import run_bass_kernel_spmd

    x = np.ascontiguousarray(np.asarray(hidden_states, np.float32)).reshape(T, H)
    router_w = np.asarray(router_w, np.float32)
    w1 = np.asarray(w1, np.float32)
    v1 = np.asarray(v1, np.float32)
    w2 = np.asarray(w2, np.float32)

    ew, ei = _route(x, router_w)
    counts = np.bincount(ei.ravel(), minlength=E)
    template, pack = _select_template(counts)
    cap = sum(template)
    nseg = len(template)
    offs = np.concatenate([[0], np.cumsum(template)]).astype(int)

    # per-expert assignment lists (token ids + weights), then cursors
    flat_e = ei.ravel()
    flat_w = ew.ravel().astype(np.float32)
    order = np.argsort(flat_e, kind="stable")
    toks_s = (order // TOPK).astype(np.int64)
    ws_s = flat_w[order]
    starts = np.concatenate([[0], np.cumsum(counts)]).astype(int)
    cursor = starts[:-1].copy()

    w1t_pre, v1t_pre, w2_pre = _prep_weights(w1, v1, w2)
    xbf = _to_bf16(x)  # [T, H] bf16

    in_maps = []
    core_lists = []  # per core: list of (seg_idx, ids) for scatter
    for c in range(NCORES):
        xt_np = np.zeros((128, 8, cap), BF16)
        w1t_np = np.zeros((nseg, NFT, 128, 8, FT), BF16)
        v1t_np = np.zeros((nseg, NFT, 128, 8, FT), BF16)
        w2_np = np.zeros((nseg, NFT, 128, 4, H), BF16)
        coef_np = np.zeros((128, cap // 128), np.float32)
        lists = []
        for s, (e, used) in sorted(pack[c].items()):
            ids = toks_s[cursor[e]:cursor[e] + used]
            ws = ws_s[cursor[e]:cursor[e] + used]
            cursor[e] += used
            st = template[s]
            L = used
            xg = np.ascontiguousarray(xbf[ids].T)     # [H, L]
            xt_np[:, :, offs[s]:offs[s] + L] = \
                xg.reshape(8, 128, L).transpose(1, 0, 2)
            wpad = np.zeros(st, np.float32)
            wpad[:L] = ws
            coef_np[:, offs[s] // 128:offs[s + 1] // 128] = \
                wpad.reshape(st // 128, 128).T
            w1t_np[s] = w1t_pre[e]
            v1t_np[s] = v1t_pre[e]
            w2_np[s] = w2_pre[e]
            lists.append((s, ids))
        core_lists.append(lists)
        in_maps.append({"xt": xt_np, "w1t": w1t_np, "v1t": v1t_np,
                        "w2": w2_np, "coef": coef_np})
    assert (cursor == starts[1:]).all()

    nc = _get_nc(template)
    if trace:
        _install_profile_shim()
    res = run_bass_kernel_spmd(nc, in_maps, list(range(NCORES)), trace=trace)

    out = np.zeros((T, H), np.float32)
    for c in range(NCORES):
        y = res.results[c]["yout"]  # [128, cap//128, H]
        yflat = y.transpose(1, 0, 2).reshape(cap, H)
        for s, ids in core_lists[c]:
            L = len(ids)
            out[ids] += yflat[offs[s]:offs[s] + L]
    return out.reshape(B, S, H), res


def kernel(hidden_states, router_w, w1, v1, w2):
    out, _ = _forward(hidden_states, router_w, w1, v1, w2, trace=False)
    return out


def _install_profile_shim():
    """The agent image's antenv lacks axon_hooks; register the NTFF
    profile hook from trn_agent_boot so trace=True works."""
    import sys
    import types
    if "antenv.axon_hooks" in sys.modules:
        return
    holder = {}
    mod = types.ModuleType("antenv.axon_hooks")
    mod.set_axon_ntff_profile_hook = lambda h: holder.__setitem__("h", h)
    mod.get_axon_ntff_profile_hook = lambda: holder.get("h")
    sys.modules["antenv.axon_hooks"] = mod
    try:
        from trn_agent_boot.trn_boot import _ntff_profile_via_ctypes
        hook = _ntff_profile_via_ctypes("/opt/axon/libaxon_pjrt.so")
        mod.set_axon_ntff_profile_hook(hook)
    except Exception as exc:  # pragma: no cover
        print(f"profile shim failed: {exc}")


# revision 6
# speedup vs baseline: 1.1623x; 1.0253x over previous
"""Mixtral MoE (top-2 of 8 experts, GLU) on 8 Trainium2 cores.

Strategy (expert-parallel, MegaBlocks-style host dispatch):
  - Host computes the router exactly (fp32, same ops as the reference) and
    flattens the T*K = 16384 (token, expert, weight) assignments.
  - Assignments are grouped by expert and packed into per-core single-expert
    "segments" chosen from a small list of static slot-size templates; the
    best (smallest-capacity) feasible template for the actual routing is
    selected at runtime and its kernel compiled once (cached). The fallback
    template [512]*5 is always feasible (sum_e ceil(c_e/512) <= 39 <= 40).
  - Each core's device inputs are host-assembled: gathered token blocks,
    per-segment pre-transposed bf16 weights, and per-token combine
    coefficients.
  - The device kernel is fully static: per segment, stream F-tiles of the
    three weight matrices, compute hmid^T = silu(w1 x^T) * (v1 x^T),
    then accumulate y = hmid @ w2 over F-tiles, scale by coef, write out.
    All matmuls are bf16 with fp32 accumulation.
  - Host scatter-adds the per-segment outputs into the full [T, H] output.
"""

import numpy as np
import ml_dtypes

B, S, H, F, E, TOPK = 4, 2048, 1024, 3584, 8, 2
T = B * S
NCORES = 8
NFT = 7                # F tiles
FT = F // NFT          # 512
BF16 = ml_dtypes.bfloat16

# Candidate per-core segment templates, preferred order (lower capacity
# first; ties prefer fewer segments = less weight streaming).
TEMPLATES = [
    (512, 512, 512, 512),            # 2048
    (1024, 1024),                    # 2048
    (512, 512, 512, 512, 128),       # 2176
    (512, 512, 512, 384, 256),       # 2176
    (768, 768, 768),                 # 2304
    (512, 512, 512, 512, 256),       # 2304
    (512, 512, 512, 384, 384),       # 2304
    (384, 384, 384, 384, 384, 384),  # 2304
    (512, 512, 512, 512, 384),       # 2432
    (512, 512, 512, 512, 512),       # 2560 — always feasible
]

_compiled = {}


# --------------------------------------------------------------------------
# device kernel
# --------------------------------------------------------------------------

def _build_nc(template):
    import concourse.tile as tile
    import concourse.mybir as mybir
    from concourse import bacc

    cap = sum(template)           # tokens per core
    nseg = len(template)
    offs = np.concatenate([[0], np.cumsum(template)]).astype(int)
    NH = H // 512                 # 512-wide h chunks
    FC = FT // 128                # 128-row f chunks per f-tile

    nc = bacc.Bacc("TRN2", target_bir_lowering=False, debug=False,
                   num_devices=NCORES)
    xt = nc.dram_tensor("xt", [128, 8, cap], mybir.dt.bfloat16,
                        kind="ExternalInput")
    w1t = nc.dram_tensor("w1t", [nseg, NFT, 128, 8, FT], mybir.dt.bfloat16,
                         kind="ExternalInput")
    v1t = nc.dram_tensor("v1t", [nseg, NFT, 128, 8, FT], mybir.dt.bfloat16,
                         kind="ExternalInput")
    w2 = nc.dram_tensor("w2", [nseg, NFT, 128, 4, H], mybir.dt.bfloat16,
                        kind="ExternalInput")
    coef = nc.dram_tensor("coef", [128, cap // 128], mybir.dt.float32,
                          kind="ExternalInput")
    yout = nc.dram_tensor("yout", [128, cap // 128, H], mybir.dt.float32,
                          kind="ExternalOutput")

    with tile.TileContext(nc) as tc:
        with (
            tc.tile_pool(name="xpool", bufs=2) as xpool,
            tc.tile_pool(name="wpool", bufs=4) as wpool,
            tc.tile_pool(name="hpool", bufs=2) as hpool,
            tc.tile_pool(name="spool", bufs=2) as spool,
            tc.tile_pool(name="opool", bufs=2) as opool,
            tc.tile_pool(name="cpool", bufs=1) as cpool,
            tc.tile_pool(name="ps1", bufs=2, space="PSUM") as ps1,
            tc.tile_pool(name="ps2", bufs=2, space="PSUM") as ps2,
            tc.tile_pool(name="pso", bufs=3, space="PSUM") as psop,
        ):
            # PE warm-up burst: independent dummy matmuls that run during
            # the initial DMA fill so HAM un-throttles before real work.
            wu = cpool.tile([128, 64], mybir.dt.bfloat16)
            nc.vector.memset(wu[:], 0.0)
            wups = ps1.tile([128, 512], mybir.dt.float32, tag="p1")
            for _ in range(90):
                nc.tensor.matmul(wups[:64, :64], wu[:, :64], wu[:, :64],
                                 start=True, stop=True)

            coefs = cpool.tile([128, cap // 128], mybir.dt.float32)
            nc.sync.dma_start(coefs[:], coef[:])
            for s in range(nseg):
                st = template[s]
                moff = offs[s] // 128       # token-block offset
                nm = st // 128              # 128-token sub-blocks
                # stage-1 token chunks (<=512 each)
                tchunks = []
                t0 = 0
                while t0 < st:
                    tl = min(512, st - t0)
                    tchunks.append((t0, tl))
                    t0 += tl

                xts = xpool.tile([128, 8, st], mybir.dt.bfloat16, tag="xts")
                nc.sync.dma_start(xts[:], xt[:, :, offs[s]:offs[s + 1]])
                oacc = opool.tile([128, nm, H], mybir.dt.float32, tag="oacc")

                for ft in range(NFT):
                    w1s = wpool.tile([128, 8, FT], mybir.dt.bfloat16, tag="w1s")
                    nc.sync.dma_start(w1s[:], w1t[s, ft])
                    v1s = wpool.tile([128, 8, FT], mybir.dt.bfloat16, tag="v1s")
                    nc.sync.dma_start(v1s[:], v1t[s, ft])
                    w2s = wpool.tile([128, 4, H], mybir.dt.bfloat16, tag="w2s")
                    nc.sync.dma_start(w2s[:], w2[s, ft])

                    hmid = hpool.tile([128, FC, st], mybir.dt.bfloat16,
                                      tag="hmid")
                    for fc in range(FC):
                        fsl = slice(fc * 128, (fc + 1) * 128)
                        for (t0, tl) in tchunks:
                            p1 = ps1.tile([128, 512], mybir.dt.float32)
                            p2 = ps2.tile([128, 512], mybir.dt.float32)
                            for hs in range(8):
                                nc.tensor.matmul(
                                    p1[:, :tl], w1s[:, hs, fsl],
                                    xts[:, hs, t0:t0 + tl],
                                    start=(hs == 0), stop=(hs == 7))
                            for hs in range(8):
                                nc.tensor.matmul(
                                    p2[:, :tl], v1s[:, hs, fsl],
                                    xts[:, hs, t0:t0 + tl],
                                    start=(hs == 0), stop=(hs == 7))
                            sil = spool.tile([128, 512], mybir.dt.float32)
                            nc.scalar.activation(
                                sil[:, :tl], p1[:, :tl],
                                mybir.ActivationFunctionType.Silu)
                            nc.vector.tensor_mul(
                                hmid[:, fc, t0:t0 + tl], sil[:, :tl],
                                p2[:, :tl])

                    for m in range(nm):
                        msl = slice(m * 128, (m + 1) * 128)
                        for n in range(NH):
                            nsl = slice(n * 512, (n + 1) * 512)
                            po = psop.tile([128, 512], mybir.dt.float32)
                            for fc in range(FC):
                                nc.tensor.matmul(
                                    po[:], hmid[:, fc, msl], w2s[:, fc, nsl],
                                    start=(fc == 0), stop=(fc == FC - 1))
                            osl = oacc[:, m, nsl]
                            if ft == 0:
                                nc.scalar.copy(osl, po[:])
                            else:
                                nc.vector.tensor_add(osl, osl, po[:])

                for m in range(nm):
                    nc.vector.tensor_scalar_mul(
                        oacc[:, m, :], oacc[:, m, :],
                        coefs[:, moff + m:moff + m + 1])
                nc.sync.dma_start(yout[:, moff:moff + nm, :], oacc[:])

    nc.compile()
    return nc


def _get_nc(template):
    if template not in _compiled:
        _compiled[template] = _build_nc(template)
    return _compiled[template]


# --------------------------------------------------------------------------
# host side: routing, packing, layout
# --------------------------------------------------------------------------

def _route(x, router_w):
    """Top-2 router, matching the reference (jax on CPU if available)."""
    try:
        import jax
        import jax.numpy as jnp
        cpu = jax.devices("cpu")[0]
        with jax.default_device(cpu):
            xl = jax.device_put(jnp.asarray(x), cpu)
            rw = jax.device_put(jnp.asarray(router_w), cpu)
            logits = xl @ rw.T
            scores = jax.nn.softmax(logits.astype(jnp.float32), axis=-1)
            ew, ei = jax.lax.top_k(scores, TOPK)
            ew = ew / ew.sum(axis=-1, keepdims=True)
            return np.asarray(ew, np.float32), np.asarray(ei, np.int64)
    except Exception:
        logits = x.astype(np.float32) @ router_w.astype(np.float32).T
        m = logits.max(axis=-1, keepdims=True)
        p = np.exp(logits - m)
        scores = (p / p.sum(axis=-1, keepdims=True)).astype(np.float32)
        i1 = scores.argmax(axis=-1)
        s2 = scores.copy()
        s2[np.arange(T), i1] = -np.inf
        i2 = s2.argmax(axis=-1)
        wa = scores[np.arange(T), i1]
        wb = scores[np.arange(T), i2]
        tot = wa + wb
        ew = np.stack([wa / tot, wb / tot], axis=-1).astype(np.float32)
        ei = np.stack([i1, i2], axis=-1).astype(np.int64)
        return ew, ei


def _try_pack(template, counts):
    """Greedy bin-pack: experts (desc count) onto 8 copies of `template`.
    Each slot holds tokens of a single expert. Returns per-core slot
    assignment [{slot_idx: (expert, n_tokens)}] or None if infeasible."""
    slots = []  # (size, core, slot_idx)
    for c in range(NCORES):
        for i, sz in enumerate(template):
            slots.append([sz, c, i, None, 0])  # size, core, idx, expert, used
    free = sorted(range(len(slots)), key=lambda i: -slots[i][0])

    for e in np.argsort(-counts):
        rem = int(counts[e])
        while rem > 0:
            # last piece: smallest free slot that fits it, else largest
            fit = [i for i in free if slots[i][0] >= rem]
            if fit:
                pick = min(fit, key=lambda i: slots[i][0])
            elif free:
                pick = free[0]
            else:
                return None
            free.remove(pick)
            take = min(rem, slots[pick][0])
            slots[pick][3] = int(e)
            slots[pick][4] = take
            rem -= take
    per_core = [dict() for _ in range(NCORES)]
    for sz, c, i, e, used in slots:
        if e is not None:
            per_core[c][i] = (e, used)
    return per_core


def _select_template(counts):
    for tpl in TEMPLATES:
        pack = _try_pack(tpl, counts)
        if pack is not None:
            return tpl, pack
    raise AssertionError("no feasible template (impossible)")


def _to_bf16(a):
    """Fast float32 -> bfloat16 with round-to-nearest-even."""
    u = np.ascontiguousarray(a, np.float32).view(np.uint32)
    r = ((u + np.uint32(0x7FFF) + ((u >> np.uint32(16)) & np.uint32(1)))
         >> np.uint32(16)).astype(np.uint16)
    return r.view(BF16)


def _prep_weights(w1, v1, w2):
    """Per-expert device layouts (bf16).

    w1t/v1t: [E][NFT,128,8,FT]  elem [ft,p,hs,f] = W[ft*FT+f, hs*128+p]
    w2     : [E][NFT,128,4,H]   elem [ft,p,fc,h] = w2[ft*FT+fc*128+p, h]
    """
    w1t, v1t, w2d = [], [], []
    for e in range(E):
        for src, dst in ((w1, w1t), (v1, v1t)):
            a = _to_bf16(src[e])                      # [F, H]
            a = np.ascontiguousarray(a.T)             # [H, F]
            a = a.reshape(8, 128, NFT, FT).transpose(2, 1, 0, 3)
            dst.append(np.ascontiguousarray(a))
        b = _to_bf16(w2[e])                           # [F, H]
        b = b.reshape(NFT, 4, 128, H).transpose(0, 2, 1, 3)
        w2d.append(np.ascontiguousarray(b))
    return w1t, v1t, w2d


def _forward(hidden_states, router_w, w1, v1, w2, trace=False):
    from concourse.bass_utils import run_bass_kernel_spmd

    x = np.ascontiguousarray(np.asarray(hidden_states, np.float32)).reshape(T, H)
    router_w = np.asarray(router_w, np.float32)
    w1 = np.asarray(w1, np.float32)
    v1 = np.asarray(v1, np.float32)
    w2 = np.asarray(w2, np.float32)

    ew, ei = _route(x, router_w)
    counts = np.bincount(ei.ravel(), minlength=E)
    template, pack = _select_template(counts)
    cap = sum(template)
    nseg = len(template)
    offs = np.concatenate([[0], np.cumsum(template)]).astype(int)

    # per-expert assignment lists (token ids + weights), then cursors
    flat_e = ei.ravel()
    flat_w = ew.ravel().astype(np.float32)
    order = np.argsort(flat_e, kind="stable")
    toks_s = (order // TOPK).astype(np.int64)
    ws_s = flat_w[order]
    starts = np.concatenate([[0], np.cumsum(counts)]).astype(int)
    cursor = starts[:-1].copy()

    w1t_pre, v1t_pre, w2_pre = _prep_weights(w1, v1, w2)
    xbf = _to_bf16(x)  # [T, H] bf16

    in_maps = []
    core_lists = []  # per core: list of (seg_idx, ids) for scatter
    for c in range(NCORES):
        xt_np = np.zeros((128, 8, cap), BF16)
        w1t_np = np.zeros((nseg, NFT, 128, 8, FT), BF16)
        v1t_np = np.zeros((nseg, NFT, 128, 8, FT), BF16)
        w2_np = np.zeros((nseg, NFT, 128, 4, H), BF16)
        coef_np = np.zeros((128, cap // 128), np.float32)
        lists = []
        for s, (e, used) in sorted(pack[c].items()):
            ids = toks_s[cursor[e]:cursor[e] + used]
            ws = ws_s[cursor[e]:cursor[e] + used]
            cursor[e] += used
            st = template[s]
            L = used
            xg = np.ascontiguousarray(xbf[ids].T)     # [H, L]
            xt_np[:, :, offs[s]:offs[s] + L] = \
                xg.reshape(8, 128, L).transpose(1, 0, 2)
            wpad = np.zeros(st, np.float32)
            wpad[:L] = ws
            coef_np[:, offs[s] // 128:offs[s + 1] // 128] = \
                wpad.reshape(st // 128, 128).T
            w1t_np[s] = w1t_pre[e]
            v1t_np[s] = v1t_pre[e]
            w2_np[s] = w2_pre[e]
            lists.append((s, ids))
        core_lists.append(lists)
        in_maps.append({"xt": xt_np, "w1t": w1t_np, "v1t": v1t_np,
                        "w2": w2_np, "coef": coef_np})
    assert (cursor == starts[1:]).all()

    nc = _get_nc(template)
    if trace:
        _install_profile_shim()
    res = run_bass_kernel_spmd(nc, in_maps, list(range(NCORES)), trace=trace)

    out = np.zeros((T, H), np.float32)
    for c in range(NCORES):
        y = res.results[c]["yout"]  # [128, cap//128, H]
        yflat = y.transpose(1, 0, 2).reshape(cap, H)
        for s, ids in core_lists[c]:
            L = len(ids)
            out[ids] += yflat[offs[s]:offs[s] + L]
    return out.reshape(B, S, H), res


def kernel(hidden_states, router_w, w1, v1, w2):
    out, _ = _forward(hidden_states, router_w, w1, v1, w2, trace=False)
    return out


def _install_profile_shim():
    """The agent image's antenv lacks axon_hooks; register the NTFF
    profile hook from trn_agent_boot so trace=True works."""
    import sys
    import types
    if "antenv.axon_hooks" in sys.modules:
        return
    holder = {}
    mod = types.ModuleType("antenv.axon_hooks")
    mod.set_axon_ntff_profile_hook = lambda h: holder.__setitem__("h", h)
    mod.get_axon_ntff_profile_hook = lambda: holder.get("h")
    sys.modules["antenv.axon_hooks"] = mod
    try:
        from trn_agent_boot.trn_boot import _ntff_profile_via_ctypes
        hook = _ntff_profile_via_ctypes("/opt/axon/libaxon_pjrt.so")
        mod.set_axon_ntff_profile_hook(hook)
    except Exception as exc:  # pragma: no cover
        print(f"profile shim failed: {exc}")


# revision 8
# speedup vs baseline: 1.1702x; 1.0068x over previous
"""Mixtral MoE (top-2 of 8 experts, GLU) on 8 Trainium2 cores.

Strategy (expert-parallel, MegaBlocks-style host dispatch):
  - Host computes the router exactly (fp32, same ops as the reference) and
    flattens the T*K = 16384 (token, expert, weight) assignments.
  - Assignments are grouped by expert and packed into per-core single-expert
    "segments" chosen from a small list of static slot-size templates; the
    best (smallest-capacity) feasible template for the actual routing is
    selected at runtime and its kernel compiled once (cached). The fallback
    template [512]*5 is always feasible (sum_e ceil(c_e/512) <= 39 <= 40).
  - Each core's device inputs are host-assembled: gathered token blocks,
    per-segment pre-transposed bf16 weights, and per-token combine
    coefficients.
  - The device kernel is fully static: per segment, stream F-tiles of the
    three weight matrices, compute hmid^T = silu(w1 x^T) * (v1 x^T),
    then accumulate y = hmid @ w2 over F-tiles, scale by coef, write out.
    All matmuls are bf16 with fp32 accumulation.
  - Host scatter-adds the per-segment outputs into the full [T, H] output.
"""

import numpy as np
import ml_dtypes

B, S, H, F, E, TOPK = 4, 2048, 1024, 3584, 8, 2
T = B * S
NCORES = 8
NFT = 7                # F tiles
FT = F // NFT          # 512
BF16 = ml_dtypes.bfloat16

# Candidate per-core segment templates, preferred order (lower capacity
# first; ties prefer fewer segments = less weight streaming).
TEMPLATES = [
    (512, 512, 512, 512),            # 2048
    (1024, 1024),                    # 2048
    (512, 512, 512, 512, 128),       # 2176
    (512, 512, 512, 384, 256),       # 2176
    (768, 768, 768),                 # 2304
    (512, 512, 512, 512, 256),       # 2304
    (512, 512, 512, 384, 384),       # 2304
    (384, 384, 384, 384, 384, 384),  # 2304
    (512, 512, 512, 512, 384),       # 2432
    (512, 512, 512, 512, 512),       # 2560 — always feasible
]

_compiled = {}


# --------------------------------------------------------------------------
# device kernel
# --------------------------------------------------------------------------

def _build_nc(template):
    import concourse.tile as tile
    import concourse.mybir as mybir
    from concourse import bacc

    cap = sum(template)           # tokens per core
    nseg = len(template)
    offs = np.concatenate([[0], np.cumsum(template)]).astype(int)
    NH = H // 512                 # 512-wide h chunks
    FC = FT // 128                # 128-row f chunks per f-tile

    nc = bacc.Bacc("TRN2", target_bir_lowering=False, debug=False,
                   num_devices=NCORES)
    xt = nc.dram_tensor("xt", [128, 8, cap], mybir.dt.bfloat16,
                        kind="ExternalInput")
    w1t = nc.dram_tensor("w1t", [nseg, NFT, 128, 8, FT], mybir.dt.bfloat16,
                         kind="ExternalInput")
    v1t = nc.dram_tensor("v1t", [nseg, NFT, 128, 8, FT], mybir.dt.bfloat16,
                         kind="ExternalInput")
    w2 = nc.dram_tensor("w2", [nseg, NFT, 128, 4, H], mybir.dt.bfloat16,
                        kind="ExternalInput")
    coef = nc.dram_tensor("coef", [128, cap // 128], mybir.dt.float32,
                          kind="ExternalInput")
    yout = nc.dram_tensor("yout", [128, cap // 128, H], mybir.dt.float32,
                          kind="ExternalOutput")

    with tile.TileContext(nc) as tc:
        with (
            tc.tile_pool(name="xpool", bufs=2) as xpool,
            tc.tile_pool(name="wpool", bufs=4) as wpool,
            tc.tile_pool(name="hpool", bufs=2) as hpool,
            tc.tile_pool(name="spool", bufs=2) as spool,
            tc.tile_pool(name="opool", bufs=2) as opool,
            tc.tile_pool(name="cpool", bufs=1) as cpool,
            tc.tile_pool(name="ps1", bufs=2, space="PSUM") as ps1,
            tc.tile_pool(name="ps2", bufs=2, space="PSUM") as ps2,
            tc.tile_pool(name="pso", bufs=3, space="PSUM") as psop,
        ):
            # PE warm-up burst: independent dummy matmuls that run during
            # the initial DMA fill so HAM un-throttles before real work.
            wu = cpool.tile([128, 128], mybir.dt.bfloat16)
            nc.vector.memset(wu[:], 0.0)
            wups = ps1.tile([128, 512], mybir.dt.float32, tag="p1")
            for _ in range(110):
                nc.tensor.matmul(wups[:, :128], wu[:], wu[:],
                                 start=True, stop=True)

            coefs = cpool.tile([128, cap // 128], mybir.dt.float32)
            nc.sync.dma_start(coefs[:], coef[:])
            for s in range(nseg):
                st = template[s]
                moff = offs[s] // 128       # token-block offset
                nm = st // 128              # 128-token sub-blocks
                # stage-1 token chunks (<=512 each)
                tchunks = []
                t0 = 0
                while t0 < st:
                    tl = min(512, st - t0)
                    tchunks.append((t0, tl))
                    t0 += tl

                xts = xpool.tile([128, 8, st], mybir.dt.bfloat16, tag="xts")
                nc.sync.dma_start(xts[:], xt[:, :, offs[s]:offs[s + 1]])
                oacc = opool.tile([128, nm, H], mybir.dt.float32, tag="oacc")

                for ft in range(NFT):
                    w1s = wpool.tile([128, 8, FT], mybir.dt.bfloat16, tag="w1s")
                    nc.sync.dma_start(w1s[:], w1t[s, ft])
                    v1s = wpool.tile([128, 8, FT], mybir.dt.bfloat16, tag="v1s")
                    nc.sync.dma_start(v1s[:], v1t[s, ft])
                    w2s = wpool.tile([128, 4, H], mybir.dt.bfloat16, tag="w2s")
                    nc.sync.dma_start(w2s[:], w2[s, ft])

                    hmid = hpool.tile([128, FC, st], mybir.dt.bfloat16,
                                      tag="hmid")
                    for fc in range(FC):
                        fsl = slice(fc * 128, (fc + 1) * 128)
                        for (t0, tl) in tchunks:
                            p1 = ps1.tile([128, 512], mybir.dt.float32)
                            p2 = ps2.tile([128, 512], mybir.dt.float32)
                            for hs in range(8):
                                nc.tensor.matmul(
                                    p1[:, :tl], w1s[:, hs, fsl],
                                    xts[:, hs, t0:t0 + tl],
                                    start=(hs == 0), stop=(hs == 7))
                            for hs in range(8):
                                nc.tensor.matmul(
                                    p2[:, :tl], v1s[:, hs, fsl],
                                    xts[:, hs, t0:t0 + tl],
                                    start=(hs == 0), stop=(hs == 7))
                            sil = spool.tile([128, 512], mybir.dt.float32)
                            nc.scalar.activation(
                                sil[:, :tl], p1[:, :tl],
                                mybir.ActivationFunctionType.Silu)
                            nc.vector.tensor_mul(
                                hmid[:, fc, t0:t0 + tl], sil[:, :tl],
                                p2[:, :tl])

                    for m in range(nm):
                        msl = slice(m * 128, (m + 1) * 128)
                        for n in range(NH):
                            nsl = slice(n * 512, (n + 1) * 512)
                            po = psop.tile([128, 512], mybir.dt.float32)
                            for fc in range(FC):
                                nc.tensor.matmul(
                                    po[:], hmid[:, fc, msl], w2s[:, fc, nsl],
                                    start=(fc == 0), stop=(fc == FC - 1))
                            osl = oacc[:, m, nsl]
                            if ft == 0:
                                nc.scalar.copy(osl, po[:])
                            else:
                                nc.vector.tensor_add(osl, osl, po[:])
                        if ft == NFT - 1:
                            # stream the finished token block out
                            nc.vector.tensor_scalar_mul(
                                oacc[:, m, :], oacc[:, m, :],
                                coefs[:, moff + m:moff + m + 1])
                            nc.sync.dma_start(yout[:, moff + m, :],
                                              oacc[:, m, :])

    nc.compile()
    return nc


def _get_nc(template):
    if template not in _compiled:
        _compiled[template] = _build_nc(template)
    return _compiled[template]


# --------------------------------------------------------------------------
# host side: routing, packing, layout
# --------------------------------------------------------------------------

def _route(x, router_w):
    """Top-2 router, matching the reference (jax on CPU if available)."""
    try:
        import jax
        import jax.numpy as jnp
        cpu = jax.devices("cpu")[0]
        with jax.default_device(cpu):
            xl = jax.device_put(jnp.asarray(x), cpu)
            rw = jax.device_put(jnp.asarray(router_w), cpu)
            logits = xl @ rw.T
            scores = jax.nn.softmax(logits.astype(jnp.float32), axis=-1)
            ew, ei = jax.lax.top_k(scores, TOPK)
            ew = ew / ew.sum(axis=-1, keepdims=True)
            return np.asarray(ew, np.float32), np.asarray(ei, np.int64)
    except Exception:
        logits = x.astype(np.float32) @ router_w.astype(np.float32).T
        m = logits.max(axis=-1, keepdims=True)
        p = np.exp(logits - m)
        scores = (p / p.sum(axis=-1, keepdims=True)).astype(np.float32)
        i1 = scores.argmax(axis=-1)
        s2 = scores.copy()
        s2[np.arange(T), i1] = -np.inf
        i2 = s2.argmax(axis=-1)
        wa = scores[np.arange(T), i1]
        wb = scores[np.arange(T), i2]
        tot = wa + wb
        ew = np.stack([wa / tot, wb / tot], axis=-1).astype(np.float32)
        ei = np.stack([i1, i2], axis=-1).astype(np.int64)
        return ew, ei


def _try_pack(template, counts):
    """Greedy bin-pack: experts (desc count) onto 8 copies of `template`.
    Each slot holds tokens of a single expert. Returns per-core slot
    assignment [{slot_idx: (expert, n_tokens)}] or None if infeasible."""
    slots = []  # (size, core, slot_idx)
    for c in range(NCORES):
        for i, sz in enumerate(template):
            slots.append([sz, c, i, None, 0])  # size, core, idx, expert, used
    free = sorted(range(len(slots)), key=lambda i: -slots[i][0])

    for e in np.argsort(-counts):
        rem = int(counts[e])
        while rem > 0:
            # last piece: smallest free slot that fits it, else largest
            fit = [i for i in free if slots[i][0] >= rem]
            if fit:
                pick = min(fit, key=lambda i: slots[i][0])
            elif free:
                pick = free[0]
            else:
                return None
            free.remove(pick)
            take = min(rem, slots[pick][0])
            slots[pick][3] = int(e)
            slots[pick][4] = take
            rem -= take
    per_core = [dict() for _ in range(NCORES)]
    for sz, c, i, e, used in slots:
        if e is not None:
            per_core[c][i] = (e, used)
    return per_core


def _select_template(counts):
    for tpl in TEMPLATES:
        pack = _try_pack(tpl, counts)
        if pack is not None:
            return tpl, pack
    raise AssertionError("no feasible template (impossible)")


def _to_bf16(a):
    """Fast float32 -> bfloat16 with round-to-nearest-even."""
    u = np.ascontiguousarray(a, np.float32).view(np.uint32)
    r = ((u + np.uint32(0x7FFF) + ((u >> np.uint32(16)) & np.uint32(1)))
         >> np.uint32(16)).astype(np.uint16)
    return r.view(BF16)


def _prep_weights(w1, v1, w2):
    """Per-expert device layouts (bf16).

    w1t/v1t: [E][NFT,128,8,FT]  elem [ft,p,hs,f] = W[ft*FT+f, hs*128+p]
    w2     : [E][NFT,128,4,H]   elem [ft,p,fc,h] = w2[ft*FT+fc*128+p, h]
    """
    w1t, v1t, w2d = [], [], []
    for e in range(E):
        for src, dst in ((w1, w1t), (v1, v1t)):
            a = _to_bf16(src[e])                      # [F, H]
            a = np.ascontiguousarray(a.T)             # [H, F]
            a = a.reshape(8, 128, NFT, FT).transpose(2, 1, 0, 3)
            dst.append(np.ascontiguousarray(a))
        b = _to_bf16(w2[e])                           # [F, H]
        b = b.reshape(NFT, 4, 128, H).transpose(0, 2, 1, 3)
        w2d.append(np.ascontiguousarray(b))
    return w1t, v1t, w2d


def _forward(hidden_states, router_w, w1, v1, w2, trace=False):
    from concourse.bass_utils import run_bass_kernel_spmd

    x = np.ascontiguousarray(np.asarray(hidden_states, np.float32)).reshape(T, H)
    router_w = np.asarray(router_w, np.float32)
    w1 = np.asarray(w1, np.float32)
    v1 = np.asarray(v1, np.float32)
    w2 = np.asarray(w2, np.float32)

    ew, ei = _route(x, router_w)
    counts = np.bincount(ei.ravel(), minlength=E)
    template, pack = _select_template(counts)
    cap = sum(template)
    nseg = len(template)
    offs = np.concatenate([[0], np.cumsum(template)]).astype(int)

    # per-expert assignment lists (token ids + weights), then cursors
    flat_e = ei.ravel()
    flat_w = ew.ravel().astype(np.float32)
    order = np.argsort(flat_e, kind="stable")
    toks_s = (order // TOPK).astype(np.int64)
    ws_s = flat_w[order]
    starts = np.concatenate([[0], np.cumsum(counts)]).astype(int)
    cursor = starts[:-1].copy()

    w1t_pre, v1t_pre, w2_pre = _prep_weights(w1, v1, w2)
    xbf = _to_bf16(x)  # [T, H] bf16

    in_maps = []
    core_lists = []  # per core: list of (seg_idx, ids) for scatter
    for c in range(NCORES):
        xt_np = np.zeros((128, 8, cap), BF16)
        w1t_np = np.zeros((nseg, NFT, 128, 8, FT), BF16)
        v1t_np = np.zeros((nseg, NFT, 128, 8, FT), BF16)
        w2_np = np.zeros((nseg, NFT, 128, 4, H), BF16)
        coef_np = np.zeros((128, cap // 128), np.float32)
        lists = []
        for s, (e, used) in sorted(pack[c].items()):
            ids = toks_s[cursor[e]:cursor[e] + used]
            ws = ws_s[cursor[e]:cursor[e] + used]
            cursor[e] += used
            st = template[s]
            L = used
            xg = np.ascontiguousarray(xbf[ids].T)     # [H, L]
            xt_np[:, :, offs[s]:offs[s] + L] = \
                xg.reshape(8, 128, L).transpose(1, 0, 2)
            wpad = np.zeros(st, np.float32)
            wpad[:L] = ws
            coef_np[:, offs[s] // 128:offs[s + 1] // 128] = \
                wpad.reshape(st // 128, 128).T
            w1t_np[s] = w1t_pre[e]
            v1t_np[s] = v1t_pre[e]
            w2_np[s] = w2_pre[e]
            lists.append((s, ids))
        core_lists.append(lists)
        in_maps.append({"xt": xt_np, "w1t": w1t_np, "v1t": v1t_np,
                        "w2": w2_np, "coef": coef_np})
    assert (cursor == starts[1:]).all()

    nc = _get_nc(template)
    if trace:
        _install_profile_shim()
    res = run_bass_kernel_spmd(nc, in_maps, list(range(NCORES)), trace=trace)

    out = np.zeros((T, H), np.float32)
    for c in range(NCORES):
        y = res.results[c]["yout"]  # [128, cap//128, H]
        yflat = y.transpose(1, 0, 2).reshape(cap, H)
        for s, ids in core_lists[c]:
            L = len(ids)
            out[ids] += yflat[offs[s]:offs[s] + L]
    return out.reshape(B, S, H), res


def kernel(hidden_states, router_w, w1, v1, w2):
    out, _ = _forward(hidden_states, router_w, w1, v1, w2, trace=False)
    return out


def _install_profile_shim():
    """The agent image's antenv lacks axon_hooks; register the NTFF
    profile hook from trn_agent_boot so trace=True works."""
    import sys
    import types
    if "antenv.axon_hooks" in sys.modules:
        return
    holder = {}
    mod = types.ModuleType("antenv.axon_hooks")
    mod.set_axon_ntff_profile_hook = lambda h: holder.__setitem__("h", h)
    mod.get_axon_ntff_profile_hook = lambda: holder.get("h")
    sys.modules["antenv.axon_hooks"] = mod
    try:
        from trn_agent_boot.trn_boot import _ntff_profile_via_ctypes
        hook = _ntff_profile_via_ctypes("/opt/axon/libaxon_pjrt.so")
        mod.set_axon_ntff_profile_hook(hook)
    except Exception as exc:  # pragma: no cover
        print(f"profile shim failed: {exc}")
